# revision 1
# baseline (speedup 1.0000x reference)
"""Expert-parallel MoE (BailingMoeV25-style) kernel for 8 trn2 NeuronCores.

Strategy:
  - Host computes routing (exact numpy replica of the reference _route) and
    packs work into a uniform SPMD grid: every core runs the same program of
    S expert-slots with per-slot block capacities caps[s]; each block is 128
    tokens through a SwiGLU MLP (H=2048 -> I=512 -> H=2048) with fp32r
    matmuls (fp32 in/out, FP22 multiply, fp32 accumulate). Only experts that
    actually receive tokens are loaded; the capacity vector is chosen at
    runtime from the observed routing (DMA/PE cost model + greedy packing).
  - The shared expert is expert -1 (identical shapes), its tokens are all
    T tokens with combine weight 1.0; routed experts' combine weights are
    pre-scaled by ROUTED_SCALING.
  - Host scatter-adds per-slot outputs back into the [T, H] result.

Program time is data-independent (fixed instruction stream), so chunk
assignment to cores only needs feasibility, not balance.
"""
import math
import sys

import numpy as np

if '/opt/trn_rl_repo' not in sys.path:
    sys.path.insert(0, '/opt/trn_rl_repo')

P = 128
T, H, E, I = 1024, 2048, 32, 512
N_KC = H // P      # 16 contraction chunks for gate/up
N_IC = I // P      # 4 chunks of the intermediate dim
N_HC = H // 512    # 4 output column chunks for down proj
TOP_K, N_GROUP, TOPK_GROUP = 4, 4, 2
ROUTED_SCALING = 2.5
N_CORES = 8

# tunables
WBUFS = 11   # gate/up weight tile slots (8KB/partition each)
XBUFS = 4
YBUFS = 2
WQ = 4       # h-chunks per weight DMA (4 -> 1MB tiles, 2 -> 0.5MB tiles)
Y_ENGINE = "sync"   # which engine queue issues y stores
X_ENGINE = "scalar"  # which engine queue issues xt loads


def route_np(x, gw, eb):
    """Exact numpy replica of reference._route (fp32)."""
    x = np.asarray(x, np.float32)
    gw = np.asarray(gw, np.float32)
    eb = np.asarray(eb, np.float32)
    logits = x @ gw.T
    scores = np.float32(1.0) / (np.float32(1.0) + np.exp(-logits, dtype=np.float32))
    sc = scores + eb[None, :]
    t, e = scores.shape
    g = e // N_GROUP
    grp = sc.reshape(t, N_GROUP, g)
    top2 = np.sort(grp, axis=-1)[:, :, -2:]
    group_scores = top2.sum(-1)
    grp_idx = np.argsort(-group_scores, kind='stable', axis=-1)[:, :TOPK_GROUP]
    gmask = np.zeros((t, N_GROUP), bool)
    gmask[np.arange(t)[:, None], grp_idx] = True
    emask = np.repeat(gmask, g, axis=1)
    masked = np.where(emask, sc, -np.inf)
    topk_ids = np.argsort(-masked, kind='stable', axis=-1)[:, :TOP_K]
    w = np.take_along_axis(scores, topk_ids, axis=1)
    w = w / w.sum(-1, keepdims=True)
    W = np.zeros((t, e), np.float32)
    np.put_along_axis(W, topk_ids, w.astype(np.float32), axis=1)
    return W


def make_plan(W):
    """Choose per-slot block capacities and assign expert chunks to core slots.

    Returns (caps, slots): caps[s] = block capacity of slot position s (same
    for every core); slots[core][s] = (expert_id, token_idx) or None.
    Expert -1 is the shared expert.
    """
    sel = W > 0
    experts = []
    for e in range(E):
        idx = np.nonzero(sel[:, e])[0]
        if len(idx):
            experts.append((e, idx))
    experts.append((-1, np.arange(T)))

    nblocks = {e: max(1, math.ceil(len(idx) / P)) for e, idx in experts}
    order = sorted(experts, key=lambda ei: -nblocks[ei[0]])

    def try_caps(caps):
        """Greedy feasibility: place each expert's blocks into free (core,slot)
        positions. Returns assignment {(core, s): (expert, n_blocks)} or None."""
        free = []  # (cap, core, s)
        for s, c in enumerate(caps):
            for core in range(N_CORES):
                free.append([c, core, s])
        placed = {}
        for e, idx in order:
            left = nblocks[e]
            while left > 0:
                cands = [f for f in free if f[0] > 0]
                if not cands:
                    return None
                # exact-fit first, else largest cap
                exact = [f for f in cands if f[0] <= left]
                f = max(exact, key=lambda f: f[0]) if exact else \
                    min(cands, key=lambda f: f[0] - left)
                take = min(left, f[0])
                placed[(f[1], f[2])] = (e, left, take)
                left -= take
                free.remove(f)
        # re-walk to record block ranges per chunk
        return placed

    # search caps vectors by increasing cost
    best = None
    for S in range(1, 7):
        import itertools
        for caps in itertools.combinations_with_replacement(range(8, 0, -1), S):
            caps = tuple(caps)
            total_cap = N_CORES * sum(caps)
            if total_cap < sum(nblocks.values()):
                continue
            t_dma = (S * 12.58 + sum(caps) * 2.10) * 2.91
            t_pe = sum(caps) * 11.5 + 4.0
            cost = max(t_dma, t_pe)
            if best is not None and cost >= best[0]:
                continue
            placed = try_caps(caps)
            if placed is None:
                continue
            best = (cost, caps, placed)
    assert best is not None, "no feasible caps vector"
    _, caps, placed = best
    S = len(caps)

    # build slots: need token ranges. Re-derive: for each expert, its chunks in
    # placement order consume its token list sequentially.
    consumed = {e: 0 for e, _ in experts}
    tokens = {e: idx for e, idx in experts}
    chunk_order = {}
    for (core, s), (e, left_before, take) in placed.items():
        chunk_order.setdefault(e, []).append((left_before, core, s, take))
    slots = [[None] * S for _ in range(N_CORES)]
    for e in tokens:
        if e not in chunk_order:
            continue
        # higher left_before = earlier chunk
        for left_before, core, s, take in sorted(chunk_order[e], key=lambda t: -t[0]):
            start = consumed[e]
            ntok = min(take * P, len(tokens[e]) - start)
            slots[core][s] = (e, tokens[e][start:start + ntok])
            consumed[e] += ntok
    return list(caps), slots


def build_program(caps):
    import concourse.bass as bass  # noqa: F401
    import concourse.mybir as mybir
    import concourse.tile as tile
    from concourse import bacc
    from concourse.masks import make_identity

    f32 = mybir.dt.float32
    f32r = mybir.dt.float32r
    AF = mybir.ActivationFunctionType

    S = len(caps)
    CB = sum(caps)              # total blocks per core
    off = [sum(caps[:s]) for s in range(S)]

    nc = bacc.Bacc()
    xt = nc.dram_tensor("xt", [CB, H, P], f32r, kind="ExternalInput")
    wv = nc.dram_tensor("wv", [P, CB], f32, kind="ExternalInput")
    wg = nc.dram_tensor("wg", [S, H, I], f32r, kind="ExternalInput")
    wu = nc.dram_tensor("wu", [S, H, I], f32r, kind="ExternalInput")
    wd = nc.dram_tensor("wd", [S, I, H], f32r, kind="ExternalInput")
    y = nc.dram_tensor("y", [CB, P, H], f32, kind="ExternalOutput")

    with tile.TileContext(nc) as tc:
        with tc.tile_pool(name="singles", bufs=1) as singles, \
             tc.tile_pool(name="wpool", bufs=WBUFS) as wpool, \
             tc.tile_pool(name="xpool", bufs=XBUFS) as xpool, \
             tc.tile_pool(name="ypool", bufs=YBUFS) as ypool, \
             tc.tile_pool(name="apool", bufs=2) as apool, \
             tc.tile_pool(name="pp", bufs=2, space="PSUM") as pp:
            ident = singles.tile([P, P], f32)
            make_identity(nc, ident)

            for s in range(S):
                wg_v = wg[s].rearrange("(c p) i -> p c i", p=P)   # [128,16,512]
                wu_v = wu[s].rearrange("(c p) i -> p c i", p=P)
                wd_v = wd[s].rearrange("(c p) h -> p c h", p=P)   # [128,4,2048]

                wg_t = []
                wu_t = []
                wd_t = []
                for q in range(16 // WQ):
                    wgt = wpool.tile([P, WQ, 512], f32r, name=f"wg_{s}_{q}", tag="w")
                    nc.sync.dma_start(out=wgt, in_=wg_v[:, WQ * q:WQ * q + WQ, :])
                    wg_t.append(wgt)
                for q in range(16 // WQ):
                    wut = wpool.tile([P, WQ, 512], f32r, name=f"wu_{s}_{q}", tag="w")
                    nc.sync.dma_start(out=wut, in_=wu_v[:, WQ * q:WQ * q + WQ, :])
                    wu_t.append(wut)
                for q in range(4):
                    wdt = wpool.tile([P, 1, H], f32r, name=f"wd_{s}_{q}", tag="wd", bufs=4)
                    nc.sync.dma_start(out=wdt, in_=wd_v[:, q:q + 1, :])
                    wd_t.append(wdt)
                wvt = singles.tile([P, caps[s]], f32, name=f"wv_{s}", tag="wv", bufs=2)
                nc.sync.dma_start(out=wvt, in_=wv[:, off[s]:off[s] + caps[s]])

                for b in range(caps[s]):
                    xtt = xpool.tile([P, N_KC, P], f32r, name=f"xt_{s}_{b}", tag="xt")
                    getattr(nc, X_ENGINE).dma_start(
                        out=xtt,
                        in_=xt[off[s] + b].rearrange("(c p) t -> p c t", p=P))

                    pg = pp.tile([P, 512], f32, name=f"pg_{s}_{b}", tag="pg")
                    pu = pp.tile([P, 512], f32, name=f"pu_{s}_{b}", tag="pu")
                    for kc in range(N_KC):
                        nc.tensor.matmul(
                            pg, xtt[:, kc, :],
                            wg_t[kc // WQ][:, kc % WQ, :],
                            start=(kc == 0), stop=(kc == N_KC - 1))
                    for kc in range(N_KC):
                        nc.tensor.matmul(
                            pu, xtt[:, kc, :],
                            wu_t[kc // WQ][:, kc % WQ, :],
                            start=(kc == 0), stop=(kc == N_KC - 1))

                    # silu(g)*u*w computed as sigmoid(g) * (u*w) * g
                    sg = apool.tile([P, 512], f32, name=f"sg_{s}_{b}", tag="sg")
                    nc.scalar.activation(sg, pg, AF.Sigmoid)
                    uw = apool.tile([P, 512], f32, name=f"uw_{s}_{b}", tag="uw")
                    nc.vector.tensor_scalar_mul(uw, pu, wvt[:, b:b + 1])
                    hh = apool.tile([P, 512], f32, name=f"hh_{s}_{b}", tag="hh")
                    nc.vector.tensor_mul(hh, sg, uw)
                    nc.vector.tensor_mul(hh, hh, pg)

                    ht = apool.tile([P, N_IC, P], f32r, name=f"ht_{s}_{b}", tag="ht")
                    for ic in range(N_IC):
                        ptr = pp.tile([P, P], f32, name=f"pt_{s}_{b}_{ic}", tag="pt")
                        nc.tensor.transpose(ptr, hh[:, ic * P:(ic + 1) * P], ident)
                        nc.vector.tensor_copy(out=ht[:, ic, :], in_=ptr)

                    ysb = ypool.tile([P, H], f32, name=f"y_{s}_{b}", tag="y")
                    for hc in range(N_HC):
                        pd = pp.tile([P, 512], f32, name=f"pd_{s}_{b}_{hc}", tag="pd")
                        for ic in range(N_IC):
                            nc.tensor.matmul(
                                pd, ht[:, ic, :],
                                wd_t[ic][:, 0, hc * 512:(hc + 1) * 512],
                                start=(ic == 0), stop=(ic == N_IC - 1))
                        nc.scalar.activation(
                            ysb[:, hc * 512:(hc + 1) * 512], pd, AF.Copy)
                    getattr(nc, Y_ENGINE).dma_start(out=y[off[s] + b], in_=ysb)
    nc.finalize()
    return nc


def pack_inputs(caps, slots, x, W, weights):
    """Build per-core input maps. weights = (w_gate, w_up, w_down, ws_gate,
    ws_up, ws_down) as fp32 numpy arrays."""
    w_gate, w_up, w_down, ws_gate, ws_up, ws_down = weights
    S = len(caps)
    CB = sum(caps)
    off = [sum(caps[:s]) for s in range(S)]
    xT = np.ascontiguousarray(np.asarray(x, np.float32).T)  # [H, T]
    in_maps = []
    for c in range(N_CORES):
        xt = np.zeros((CB, H, P), np.float32)
        wvv = np.zeros((P, CB), np.float32)
        wgv = np.zeros((S, H, I), np.float32)
        wuv = np.zeros((S, H, I), np.float32)
        wdv = np.zeros((S, I, H), np.float32)
        for s in range(S):
            ch = slots[c][s]
            if ch is None:
                continue
            e, idx = ch
            if e == -1:
                wgv[s] = ws_gate
                wuv[s] = ws_up
                wdv[s] = ws_down
                wts = np.ones(len(idx), np.float32)
            else:
                wgv[s] = w_gate[e]
                wuv[s] = w_up[e]
                wdv[s] = w_down[e]
                wts = W[idx, e] * np.float32(ROUTED_SCALING)
            for b in range(caps[s]):
                blk = idx[b * P:(b + 1) * P]
                if len(blk) == 0:
                    break
                xt[off[s] + b, :, :len(blk)] = xT[:, blk]
                wvv[:len(blk), off[s] + b] = wts[b * P:(b + 1) * P]
        in_maps.append({"xt": xt, "wv": wvv, "wg": wgv, "wu": wuv, "wd": wdv})
    return in_maps


def combine(caps, slots, results):
    S = len(caps)
    off = [sum(caps[:s]) for s in range(S)]
    out = np.zeros((T, H), np.float32)
    for c in range(N_CORES):
        yv = results[c]["y"]
        for s in range(S):
            ch = slots[c][s]
            if ch is None:
                continue
            _, idx = ch
            for b in range(caps[s]):
                blk = idx[b * P:(b + 1) * P]
                if len(blk) == 0:
                    break
                out[blk] += yv[off[s] + b, :len(blk)]
    return out


def prepare(**inputs):
    """Routing + planning + packing (everything except device execution)."""
    x = np.asarray(inputs["hidden_states"], np.float32)
    W = route_np(x, inputs["gate_w"], inputs["expert_bias"])
    caps, slots = make_plan(W)
    weights = tuple(
        np.asarray(inputs[k], np.float32)
        for k in ("w_gate", "w_up", "w_down", "ws_gate", "ws_up", "ws_down"))
    in_maps = pack_inputs(caps, slots, x, W, weights)
    return caps, slots, in_maps


def kernel(**inputs):
    from concourse.bass_utils import run_bass_kernel_spmd
    caps, slots, in_maps = prepare(**inputs)
    nc = build_program(caps)
    res = run_bass_kernel_spmd(nc, in_maps, core_ids=list(range(N_CORES)))
    return combine(caps, slots, res.results)



# revision 38
# speedup vs baseline: 1.6844x; 1.6844x over previous
"""Expert-parallel MoE (BailingMoeV25-style) kernel for 8 trn2 NeuronCores.

v2 strategy — token-moving layout:
  - Host computes routing (exact numpy replica of the reference _route).
  - Device math is x^T-side: for each expert "slot" the weights are the
    matmul stationary operand and TOKENS are the moving dim, so PE time
    scales with actual routed tokens instead of padded 128-token blocks.
      g^T[I,R]  = wg^T chunks @ x^T      (64 matmuls of R rows)
      u^T[I,R]  = wu^T chunks @ (w*x)^T  (combine weight folded into xu on host)
      h^T       = silu(g^T) * u^T        (Act Silu + DVE mul, bf16)
      y^T[H,R]  = wd^T chunks @ h^T      (64 matmuls of R rows)
  - All matmul operands are bf16 (full PE rate in the cost model; halves
    DMA bytes vs fp32); accumulation stays fp32 in PSUM.
  - Uniform SPMD program: S expert-slots, each with fixed run sizes
    (runs share the slot's single weight load). Host packs tokens by
    expert into runs; padded columns are zeros.
  - DMA is spread over the 3 DMA-capable queues (sync/SP, scalar/Act,
    gpsimd/Pool) which the cost model executes in parallel:
      sync:   wg + y      scalar: wu + xg      gpsimd: wd + xu
  - Host scatter-adds per-run y^T back into the [T, H] result.
"""
import itertools
import sys

import numpy as np

if '/opt/trn_rl_repo' not in sys.path:
    sys.path.insert(0, '/opt/trn_rl_repo')

import ml_dtypes

BF16 = ml_dtypes.bfloat16

P = 128
T, H, E, I = 1024, 2048, 32, 512
N_KC = H // P      # 16 contraction chunks for gate/up
N_IC = I // P      # 4 chunks of the intermediate dim
N_HC = H // P      # 16 output chunks for down proj
TOP_K, N_GROUP, TOPK_GROUP = 4, 4, 2
ROUTED_SCALING = 2.5
N_CORES = 8
RMAX = 512         # max run size (one PSUM bank of fp32)


def route_np(x, gw, eb):
    """Exact numpy replica of reference._route (fp32)."""
    x = np.asarray(x, np.float32)
    gw = np.asarray(gw, np.float32)
    eb = np.asarray(eb, np.float32)
    logits = x @ gw.T
    scores = np.float32(1.0) / (np.float32(1.0) + np.exp(-logits, dtype=np.float32))
    sc = scores + eb[None, :]
    t, e = scores.shape
    g = e // N_GROUP
    grp = sc.reshape(t, N_GROUP, g)
    top2 = np.sort(grp, axis=-1)[:, :, -2:]
    group_scores = top2.sum(-1)
    grp_idx = np.argsort(-group_scores, kind='stable', axis=-1)[:, :TOPK_GROUP]
    gmask = np.zeros((t, N_GROUP), bool)
    gmask[np.arange(t)[:, None], grp_idx] = True
    emask = np.repeat(gmask, g, axis=1)
    masked = np.where(emask, sc, -np.inf)
    topk_ids = np.argsort(-masked, kind='stable', axis=-1)[:, :TOP_K]
    w = np.take_along_axis(scores, topk_ids, axis=1)
    w = w / w.sum(-1, keepdims=True)
    W = np.zeros((t, e), np.float32)
    np.put_along_axis(W, topk_ids, w.astype(np.float32), axis=1)
    return W


# cost-model constants (CoreSim calibration)
NS_PER_ROW = 80.0          # 192 matmul-rows/token at bf16, 0.4167 ns/row
NS_PER_RUN = 1200.0        # act/vec + sem overhead per run
NS_PER_SLOT = 2600.0       # weight-load serialization (prefetch depth 1)
NS_PER_BYTE_Q = 3.06e-3    # per-queue DMA (327 GB/s effective)
NS_PER_DMA = 1400.0        # per-DMA queue-side overhead
W_BYTES = H * I * 2        # one bf16 weight matrix (2 MB)


def plan_cost(struct):
    S = len(struct)
    rows = sum(sum(s) for s in struct)
    nruns = sum(len(s) for s in struct)
    pe = rows * NS_PER_ROW + nruns * NS_PER_RUN + S * NS_PER_SLOT + 4000.0
    xb = rows * H * 2
    q = S * W_BYTES + xb               # each queue: one weight stream + x/y
    dma = q * NS_PER_BYTE_Q + (S + nruns) * NS_PER_DMA
    return max(pe, dma) + 0.05 * (pe + dma)


def make_plan(W, force=None):
    """Choose a uniform slot/run structure and assign expert chunks.

    Returns (struct, slots): struct[s] = tuple of run sizes for slot s
    (same on every core); slots[core][s] = (expert_id, token_idx) or None.
    Expert -1 is the shared expert.
    """
    sel = W > 0
    experts = []
    for e in range(E):
        idx = np.nonzero(sel[:, e])[0]
        if len(idx):
            experts.append((e, idx))
    experts.append((-1, np.arange(T)))
    sizes = {e: len(idx) for e, idx in experts}
    order = sorted(experts, key=lambda ei: -sizes[ei[0]])
    total = sum(sizes.values())

    def _place(struct, rule):
        caps = [sum(s) for s in struct]
        free = [[c, core, s] for s, c in enumerate(caps) for core in range(N_CORES)]
        placed = {}
        for e, idx in order:
            left = sizes[e]
            while left > 0:
                if not free:
                    return None
                if rule == 'snug':
                    geq = [f for f in free if f[0] >= left]
                    f = min(geq, key=lambda f: f[0]) if geq else \
                        max(free, key=lambda f: f[0])
                elif rule == 'big':
                    f = max(free, key=lambda f: f[0])
                else:
                    exact = [f for f in free if f[0] <= left]
                    f = max(exact, key=lambda f: f[0]) if exact else \
                        min(free, key=lambda f: f[0] - left)
                take = min(left, f[0])
                placed[(f[1], f[2])] = (e, left, take)
                left -= take
                free.remove(f)
        return placed

    def try_struct(struct):
        for rule in ('snug', 'big', 'legacy'):
            placed = _place(struct, rule)
            if placed is not None:
                return placed
        return None

    MENU = [(512,), (448,), (384,), (320,), (256,), (224,), (192,), (160,),
            (128,), (96,), (64,), (48,), (32,),
            (512, 128), (512, 96), (512, 64), (512, 32), (512, 192),
            (512, 160), (256, 64), (256, 32), (192, 64), (128, 32)]

    if force is not None:
        placed = try_struct(force)
        assert placed is not None, f"forced struct {force} infeasible"
        best = (0.0, tuple(force), placed)
    else:
        best = None
    for S in (() if force is not None else range(1, 6)):
        for struct in itertools.combinations_with_replacement(MENU, S):
            if N_CORES * sum(sum(s) for s in struct) < total:
                continue
            c = plan_cost(struct)
            if best is not None and c >= best[0]:
                continue
            placed = try_struct(struct)
            if placed is None:
                continue
            best = (c, struct, placed)
    if best is None:
        # fallback: grow generic structures until feasible
        for k in range(5, 40):
            struct = tuple([(RMAX,)] * 2 + [(192,)] * k)
            placed = try_struct(struct)
            if placed is not None:
                best = (plan_cost(struct), struct, placed)
                break
    assert best is not None, "no feasible structure"
    _, struct, placed = best
    S = len(struct)

    # token ranges: each expert's chunks consume its token list sequentially
    consumed = {e: 0 for e, _ in experts}
    tokens = {e: idx for e, idx in experts}
    chunk_order = {}
    for (core, s), (e, left_before, take) in placed.items():
        chunk_order.setdefault(e, []).append((left_before, core, s, take))
    slots = [[None] * S for _ in range(N_CORES)]
    for e in tokens:
        if e not in chunk_order:
            continue
        for left_before, core, s, take in sorted(chunk_order[e], key=lambda t: -t[0]):
            start = consumed[e]
            slots[core][s] = (e, tokens[e][start:start + take])
            consumed[e] += take
    return list(struct), slots


def build_program(struct):
    import concourse.bass as bass  # noqa: F401
    import concourse.mybir as mybir
    import concourse.tile as tile
    from concourse import bacc

    f32 = mybir.dt.float32
    bf16 = mybir.dt.bfloat16
    AF = mybir.ActivationFunctionType

    S = len(struct)
    TOT = sum(sum(s) for s in struct)
    # flat column offsets per (slot, run)
    offs = []
    o = 0
    for s in range(S):
        ro = []
        for r in struct[s]:
            ro.append(o)
            o += r
        offs.append(ro)

    nc = bacc.Bacc()
    xg = nc.dram_tensor("xg", [P, N_KC, TOT], bf16, kind="ExternalInput")
    wv = nc.dram_tensor("wv", [1, TOT], f32, kind="ExternalInput")
    wgt = nc.dram_tensor("wgt", [S, P, N_KC, I], bf16, kind="ExternalInput")
    wut = nc.dram_tensor("wut", [S, P, N_KC, I], bf16, kind="ExternalInput")
    wdt = nc.dram_tensor("wdt", [S, P, N_IC, H], bf16, kind="ExternalInput")
    y = nc.dram_tensor("y", [P, N_KC, TOT], bf16, kind="ExternalOutput")

    # flatten runs; software-pipeline: gate/up of run j, then down of run j-1,
    # so the PE never idles (idle resets the p-state ramp to 2x cycle time).
    runs = []
    for s in range(S):
        for r, R in enumerate(struct[s]):
            runs.append((s, r, R, offs[s][r]))

    with tile.TileContext(nc) as tc:
        with tc.tile_pool(name="singles", bufs=1) as singles, \
             tc.tile_pool(name="wpool", bufs=2) as wpool, \
             tc.tile_pool(name="xpool", bufs=2) as xpool, \
             tc.tile_pool(name="hpool", bufs=2) as hpool, \
             tc.tile_pool(name="ypool", bufs=1) as ypool, \
             tc.tile_pool(name="pp", bufs=2, space="PSUM") as pp:
            w_tiles = {}
            state = {}  # pipeline state of the previous run
            queues = [nc.sync, nc.scalar, nc.gpsimd]
            KC3 = [(0, 6), (6, 11), (11, 16)]
            H3 = [(0, 640), (640, 1280), (1280, H)]

            # separate tile per DMA chunk: consumers depend on the chunk that
            # carries their slice, not on the whole-matrix load
            def emit_wg(s):
                ts = []
                for q, (a, b) in enumerate(KC3):
                    t = wpool.tile([P, b - a, I], bf16, name=f"wg_{s}_{q}",
                                   tag=f"wg{q}")
                    queues[(q + 1) % 3].dma_start(out=t, in_=wgt[s][:, a:b, :])
                    ts.append(((a, b), t))
                return ts

            def emit_wu(s):
                ts = []
                for q, (a, b) in enumerate(KC3):
                    t = wpool.tile([P, b - a, I], bf16, name=f"wu_{s}_{q}",
                                   tag=f"wu{q}")
                    queues[(q + 2) % 3].dma_start(out=t, in_=wut[s][:, a:b, :])
                    ts.append(((a, b), t))
                return ts

            def emit_wd(s):
                ts = []
                for q, (a, b) in enumerate(H3):
                    t = wpool.tile([P, N_IC, b - a], bf16, name=f"wd_{s}_{q}",
                                   tag=f"wd{q}")
                    queues[q].dma_start(out=t, in_=wdt[s][:, :, a:b])
                    ts.append(((a, b), t))
                return ts

            def emit_weights(s):
                w_tiles[s] = (emit_wg(s), emit_wu(s), emit_wd(s))

            def kc_slice(ts, kc):
                for (a, b), t in ts:
                    if a <= kc < b:
                        return t[:, kc - a, :]
                raise AssertionError

            def hc_slice(ts, ic, hc):
                for (a, b), t in ts:
                    if a <= hc * P < b:
                        return t[:, ic, hc * P - a:(hc + 1) * P - a]
                raise AssertionError

            def emit_down(prev, last=False):
                s, r, R, O = prev["run"]
                wd_t = w_tiles[s][2]
                h_t = prev["h"]
                YB = [(0, 6), (6, 11), (11, 16)]
                y_ts = [ypool.tile([P, b - a, RMAX], bf16, name=f"y_{s}_{r}_{q}",
                                   tag=f"y{q}")
                        for q, (a, b) in enumerate(YB)]
                for hc in range(N_HC):
                    pd = pp.tile([P, RMAX], f32, name=f"pd_{s}_{r}_{hc}",
                                 tag="pd", bufs=3)
                    for ic in range(N_IC):
                        nc.tensor.matmul(
                            pd[:, :R],
                            hc_slice(wd_t, ic, hc),
                            h_t[ic][:, :R],
                            start=(ic == 0), stop=(ic == N_IC - 1))
                    yq = 0 if hc < 6 else (1 if hc < 11 else 2)
                    ya = YB[yq][0]
                    if hc % 2 == 0:
                        nc.vector.tensor_copy(out=y_ts[yq][:, hc - ya, :R],
                                              in_=pd[:, :R])
                    else:
                        nc.scalar.activation(y_ts[yq][:, hc - ya, :R],
                                             pd[:, :R], AF.Copy)
                    if hc + 1 in (6, 11, 16):
                        a, b = YB[yq]
                        queues[yq].dma_start(out=y[:, a:b, O:O + R],
                                             in_=y_ts[yq][:, :, :R])

            def emit_xg(s, r, R, O):
                ts = []
                for q, (a, b) in enumerate(KC3):
                    t = xpool.tile([P, b - a, RMAX], bf16,
                                   name=f"xg_{s}_{r}_{q}", tag=f"xg{q}")
                    queues[q].dma_start(out=t[:, :, :R], in_=xg[:, a:b, O:O + R])
                    ts.append(((a, b), t))
                return ts

            # combine-weight vector, broadcast on device per run
            wv_t = singles.tile([1, TOT], f32, name="wv", tag="wv")
            nc.sync.dma_start(out=wv_t, in_=wv[:, :])

            # startup: stream gate operands in fine kc pieces, round-robin on
            # queues, xg piece adjacent to wg piece so the first matmuls can
            # begin as soon as the first pair lands
            s0, r0, R0, O0 = runs[0]
            x_tiles = {}
            KCF = [(0, 1), (1, 2), (2, 3), (3, 5), (5, 7), (7, 9), (9, 12),
                   (12, 16)]
            xg0, wg0 = [], []
            for k, (a, b) in enumerate(KCF):
                q = queues[k % 3]
                xt = xpool.tile([P, b - a, RMAX], bf16, name=f"xg_f_{k}",
                                tag=f"xgf{k}", bufs=1)
                q.dma_start(out=xt[:, :, :R0], in_=xg[:, a:b, O0:O0 + R0])
                xg0.append(((a, b), xt))
                wt = wpool.tile([P, b - a, I], bf16, name=f"wg_f_{k}",
                                tag=f"wgf{k}", bufs=1)
                q.dma_start(out=wt, in_=wgt[0][:, a:b, :])
                wg0.append(((a, b), wt))
            w_tiles[0] = (wg0, emit_wu(0), emit_wd(0))
            x_tiles[0] = xg0

            # gate/up interleave tolerant of late wu arrival; first run is
            # all-gate kc-outer so matmuls start with the first piece pair
            GU = [(0, 'g'), (1, 'g'), (0, 'u'), (2, 'g'), (1, 'u'),
                  (3, 'g'), (2, 'u'), (3, 'u')]
            GU0 = [(0, 'g'), (1, 'g'), (2, 'g'), (3, 'g'),
                   (0, 'u'), (1, 'u'), (2, 'u'), (3, 'u')]

            for j, (s, r, R, O) in enumerate(runs):
                if j not in x_tiles:
                    x_tiles[j] = emit_xg(s, r, R, O)
                xg_t = x_tiles.pop(j)
                # prefetch next slot's weights one full slot ahead
                if r == 0 and s + 1 < S:
                    emit_weights(s + 1)

                wb = hpool.tile([P, RMAX], f32, name=f"wb_{s}_{r}", tag="wb")
                nc.gpsimd.partition_broadcast(wb[:, :R], wv_t[:, O:O + R])

                wg_t, wu_t, _ = w_tiles[s]
                pg_t = [None] * N_IC
                h_t = [None] * N_IC
                for ic, which in GU:
                    if which == 'g':
                        pg = pp.tile([P, RMAX], f32, name=f"pg_{s}_{r}_{ic}",
                                     tag="pg", bufs=3)
                        for kc in range(N_KC):
                            nc.tensor.matmul(
                                pg[:, :R],
                                kc_slice(wg_t, kc)[:, ic * P:(ic + 1) * P],
                                kc_slice(xg_t, kc)[:, :R],
                                start=(kc == 0), stop=(kc == N_KC - 1))
                        sil = hpool.tile([P, RMAX], f32,
                                         name=f"sil_{s}_{r}_{ic}", tag="sil")
                        nc.scalar.activation(sil[:, :R], pg[:, :R], AF.Sigmoid)
                        silg = hpool.tile([P, RMAX], f32,
                                          name=f"silg_{s}_{r}_{ic}", tag="silg")
                        nc.vector.tensor_mul(silg[:, :R], sil[:, :R], pg[:, :R])
                        pg_t[ic] = silg
                    else:
                        pu = pp.tile([P, RMAX], f32, name=f"pu_{s}_{r}_{ic}",
                                     tag="pu")
                        for kc in range(N_KC):
                            nc.tensor.matmul(
                                pu[:, :R],
                                kc_slice(wu_t, kc)[:, ic * P:(ic + 1) * P],
                                kc_slice(xg_t, kc)[:, :R],
                                start=(kc == 0), stop=(kc == N_KC - 1))
                        silg = pg_t[ic]
                        uw = hpool.tile([P, RMAX], f32, name=f"uw_{s}_{r}_{ic}",
                                        tag="uw")
                        nc.vector.tensor_mul(uw[:, :R], silg[:, :R], pu[:, :R])
                        ht = hpool.tile([P, RMAX], bf16, name=f"h_{s}_{r}_{ic}",
                                        tag=f"h{ic}")
                        nc.vector.tensor_mul(ht[:, :R], uw[:, :R], wb[:, :R])
                        h_t[ic] = ht

                if state:
                    emit_down(state)
                state = {"run": (s, r, R, O), "h": h_t}
            emit_down(state, last=True)
    nc.finalize()
    return nc


def pack_inputs(struct, slots, x, W, weights):
    """Build per-core input maps (bf16)."""
    w_gate, w_up, w_down, ws_gate, ws_up, ws_down = weights
    S = len(struct)
    TOT = sum(sum(s) for s in struct)
    offs = []
    o = 0
    for s in range(S):
        ro = []
        for r in struct[s]:
            ro.append(o)
            o += r
        offs.append(ro)

    x = np.asarray(x, np.float32)
    # xTr[p, c, t] = x[t, c*128+p]
    xTr = np.ascontiguousarray(x.T.reshape(N_KC, P, T).transpose(1, 0, 2))
    xTr16 = xTr.astype(BF16)

    wcache = {}

    def expert_w(e):
        if e not in wcache:
            if e == -1:
                g, u, d = ws_gate, ws_up, ws_down
            else:
                g, u, d = w_gate[e], w_up[e], w_down[e]
            wcache[e] = (
                np.ascontiguousarray(
                    g.reshape(N_KC, P, I).transpose(1, 0, 2)).astype(BF16),
                np.ascontiguousarray(
                    u.reshape(N_KC, P, I).transpose(1, 0, 2)).astype(BF16),
                np.ascontiguousarray(
                    d.reshape(N_IC, P, H).transpose(1, 0, 2)).astype(BF16),
            )
        return wcache[e]

    in_maps = []
    for c in range(N_CORES):
        xgv = np.zeros((P, N_KC, TOT), BF16)
        wvv = np.zeros((1, TOT), np.float32)
        wgv = np.zeros((S, P, N_KC, I), BF16)
        wuv = np.zeros((S, P, N_KC, I), BF16)
        wdv = np.zeros((S, P, N_IC, H), BF16)
        for s in range(S):
            ch = slots[c][s]
            if ch is None:
                continue
            e, idx = ch
            wgv[s], wuv[s], wdv[s] = expert_w(e)
            if e == -1:
                wts = np.ones(len(idx), np.float32)
            else:
                wts = W[idx, e] * np.float32(ROUTED_SCALING)
            pos = 0
            for r, R in enumerate(struct[s]):
                blk = idx[pos:pos + R]
                if len(blk) == 0:
                    break
                O = offs[s][r]
                xgv[:, :, O:O + len(blk)] = xTr16[:, :, blk]
                wvv[0, O:O + len(blk)] = wts[pos:pos + len(blk)]
                pos += len(blk)
        in_maps.append({"xg": xgv, "wv": wvv, "wgt": wgv, "wut": wuv,
                        "wdt": wdv})
    return in_maps


def combine(struct, slots, results):
    S = len(struct)
    offs = []
    o = 0
    for s in range(S):
        ro = []
        for r in struct[s]:
            ro.append(o)
            o += r
        offs.append(ro)
    out = np.zeros((T, H), np.float32)
    for c in range(N_CORES):
        yv = np.asarray(results[c]["y"], dtype=np.float32)  # [P, 16, TOT]
        yfull = yv.transpose(1, 0, 2).reshape(H, -1)        # [H, TOT]
        for s in range(S):
            ch = slots[c][s]
            if ch is None:
                continue
            _, idx = ch
            pos = 0
            for r, R in enumerate(struct[s]):
                blk = idx[pos:pos + R]
                if len(blk) == 0:
                    break
                O = offs[s][r]
                out[blk] += yfull[:, O:O + len(blk)].T
                pos += len(blk)
    return out


def prepare(**inputs):
    """Routing + planning + packing (everything except device execution)."""
    x = np.asarray(inputs["hidden_states"], np.float32)
    W = route_np(x, inputs["gate_w"], inputs["expert_bias"])
    struct, slots = make_plan(W)
    weights = tuple(
        np.asarray(inputs[k], np.float32)
        for k in ("w_gate", "w_up", "w_down", "ws_gate", "ws_up", "ws_down"))
    in_maps = pack_inputs(struct, slots, x, W, weights)
    return struct, slots, in_maps


def kernel(**inputs):
    from concourse.bass_utils import run_bass_kernel_spmd
    struct, slots, in_maps = prepare(**inputs)
    nc = build_program(struct)
    res = run_bass_kernel_spmd(nc, in_maps, core_ids=list(range(N_CORES)))
    return combine(struct, slots, res.results)


# revision 47
# speedup vs baseline: 1.7253x; 1.0243x over previous
"""Expert-parallel MoE (BailingMoeV25-style) kernel for 8 trn2 NeuronCores.

v2 strategy — token-moving layout:
  - Host computes routing (exact numpy replica of the reference _route).
  - Device math is x^T-side: for each expert "slot" the weights are the
    matmul stationary operand and TOKENS are the moving dim, so PE time
    scales with actual routed tokens instead of padded 128-token blocks.
      g^T[I,R]  = wg^T chunks @ x^T      (64 matmuls of R rows)
      u^T[I,R]  = wu^T chunks @ x^T
      h^T       = silu(g^T) * u^T        (Act Silu + DVE mul, bf16)
      y^T[H,R]  = wd^T chunks @ h^T      (64 matmuls of R rows)
  - All matmul operands are bf16 (full PE rate in the cost model; halves
    DMA bytes vs fp32); accumulation stays fp32 in PSUM.
  - Uniform SPMD program: S expert-slots, each with fixed run sizes
    (runs share the slot's single weight load). Host packs tokens by
    expert into runs; padded columns are zeros.
  - DMA is spread over the 3 DMA-capable queues (sync/SP, scalar/Act,
    gpsimd/Pool) which the cost model executes in parallel:
      every stream (weights, x, y) is split in thirds across the queues
  - Host scatter-adds per-run y^T back into the [T, H] result.
"""
import itertools
import sys

import numpy as np

if '/opt/trn_rl_repo' not in sys.path:
    sys.path.insert(0, '/opt/trn_rl_repo')

import ml_dtypes

BF16 = ml_dtypes.bfloat16

P = 128
T, H, E, I = 1024, 2048, 32, 512
N_KC = H // P      # 16 contraction chunks for gate/up
N_IC = I // P      # 4 chunks of the intermediate dim
N_HC = H // P      # 16 output chunks for down proj
TOP_K, N_GROUP, TOPK_GROUP = 4, 4, 2
ROUTED_SCALING = 2.5
N_CORES = 8
RMAX = 512         # max run size (one PSUM bank of fp32)
WARM = 68          # dummy matmuls bridging the initial DMA wait (PE warmup)


def route_np(x, gw, eb):
    """Exact numpy replica of reference._route (fp32)."""
    x = np.asarray(x, np.float32)
    gw = np.asarray(gw, np.float32)
    eb = np.asarray(eb, np.float32)
    logits = x @ gw.T
    scores = np.float32(1.0) / (np.float32(1.0) + np.exp(-logits, dtype=np.float32))
    sc = scores + eb[None, :]
    t, e = scores.shape
    g = e // N_GROUP
    grp = sc.reshape(t, N_GROUP, g)
    top2 = np.sort(grp, axis=-1)[:, :, -2:]
    group_scores = top2.sum(-1)
    grp_idx = np.argsort(-group_scores, kind='stable', axis=-1)[:, :TOPK_GROUP]
    gmask = np.zeros((t, N_GROUP), bool)
    gmask[np.arange(t)[:, None], grp_idx] = True
    emask = np.repeat(gmask, g, axis=1)
    masked = np.where(emask, sc, -np.inf)
    topk_ids = np.argsort(-masked, kind='stable', axis=-1)[:, :TOP_K]
    w = np.take_along_axis(scores, topk_ids, axis=1)
    w = w / w.sum(-1, keepdims=True)
    W = np.zeros((t, e), np.float32)
    np.put_along_axis(W, topk_ids, w.astype(np.float32), axis=1)
    return W


# cost-model constants (calibrated against CoreSim traces)
NS_PER_ROW = 82.0          # 192 matmul-rows/token at bf16, 0.4167 ns/row
NS_PER_RUN = 800.0         # act/vec + sem overhead per run
NS_PER_SLOT = 1000.0       # residual slot-boundary exposure (prefetched)
NS_PER_BYTE_Q = 3.06e-3    # per-queue DMA (327 GB/s effective)
NS_PER_DMA = 1400.0        # per-DMA queue-side overhead
W_BYTES = H * I * 2        # one bf16 weight matrix (2 MB)


def plan_cost(struct):
    S = len(struct)
    rows = sum(sum(s) for s in struct)
    nruns = sum(len(s) for s in struct)
    pe = rows * NS_PER_ROW + nruns * NS_PER_RUN + S * NS_PER_SLOT + 8000.0
    # per queue: one third of each weight matrix per slot + a third of x and y
    q = S * W_BYTES + rows * H * 2 * 2 // 3
    dma = q * NS_PER_BYTE_Q + (S * 3 + nruns * 2) * NS_PER_DMA
    return max(pe, dma) + 0.05 * (pe + dma)


def make_plan(W, force=None):
    """Choose a uniform slot/run structure and assign expert chunks.

    Returns (struct, slots): struct[s] = tuple of run sizes for slot s
    (same on every core); slots[core][s] = (expert_id, token_idx) or None.
    Expert -1 is the shared expert.
    """
    sel = W > 0
    experts = []
    for e in range(E):
        idx = np.nonzero(sel[:, e])[0]
        if len(idx):
            experts.append((e, idx))
    experts.append((-1, np.arange(T)))
    sizes = {e: len(idx) for e, idx in experts}
    order = sorted(experts, key=lambda ei: -sizes[ei[0]])
    total = sum(sizes.values())

    def _place(struct, rule):
        caps = [sum(s) for s in struct]
        free = [[c, core, s] for s, c in enumerate(caps) for core in range(N_CORES)]
        placed = {}
        for e, idx in order:
            left = sizes[e]
            while left > 0:
                if not free:
                    return None
                if rule == 'snug':
                    geq = [f for f in free if f[0] >= left]
                    f = min(geq, key=lambda f: f[0]) if geq else \
                        max(free, key=lambda f: f[0])
                elif rule == 'big':
                    f = max(free, key=lambda f: f[0])
                else:
                    exact = [f for f in free if f[0] <= left]
                    f = max(exact, key=lambda f: f[0]) if exact else \
                        min(free, key=lambda f: f[0] - left)
                take = min(left, f[0])
                placed[(f[1], f[2])] = (e, left, take)
                left -= take
                free.remove(f)
        return placed

    def try_struct(struct):
        for rule in ('snug', 'big', 'legacy'):
            placed = _place(struct, rule)
            if placed is not None:
                return placed
        return None

    MENU = [(512,), (448,), (384,), (320,), (256,), (224,), (192,), (160,),
            (128,), (96,), (64,), (48,), (32,),
            (512, 128), (512, 96), (512, 64), (512, 32), (512, 192),
            (512, 160), (256, 64), (256, 32), (192, 64), (128, 32)]

    if force is not None:
        placed = try_struct(force)
        assert placed is not None, f"forced struct {force} infeasible"
        best = (0.0, tuple(force), placed)
    else:
        # cost is exact given the structure, so walking candidates in
        # ascending cost and taking the first feasible one is optimal
        cands = []
        for S in range(1, 6):
            for struct in itertools.combinations_with_replacement(MENU, S):
                if N_CORES * sum(sum(s) for s in struct) < total:
                    continue
                cands.append((plan_cost(struct), struct))
        cands.sort(key=lambda cs: cs[0])
        best = None
        for c, struct in cands:
            placed = try_struct(struct)
            if placed is not None:
                best = (c, struct, placed)
                break
    if best is None:
        # fallback: grow generic structures until feasible
        for k in range(5, 40):
            struct = tuple([(RMAX,)] * 2 + [(192,)] * k)
            placed = try_struct(struct)
            if placed is not None:
                best = (plan_cost(struct), struct, placed)
                break
    assert best is not None, "no feasible structure"
    _, struct, placed = best
    S = len(struct)

    # token ranges: each expert's chunks consume its token list sequentially
    consumed = {e: 0 for e, _ in experts}
    tokens = {e: idx for e, idx in experts}
    chunk_order = {}
    for (core, s), (e, left_before, take) in placed.items():
        chunk_order.setdefault(e, []).append((left_before, core, s, take))
    slots = [[None] * S for _ in range(N_CORES)]
    for e in tokens:
        if e not in chunk_order:
            continue
        for left_before, core, s, take in sorted(chunk_order[e], key=lambda t: -t[0]):
            start = consumed[e]
            slots[core][s] = (e, tokens[e][start:start + take])
            consumed[e] += take
    return list(struct), slots


def build_program(struct):
    import concourse.bass as bass  # noqa: F401
    import concourse.mybir as mybir
    import concourse.tile as tile
    from concourse import bacc

    f32 = mybir.dt.float32
    bf16 = mybir.dt.bfloat16
    AF = mybir.ActivationFunctionType

    S = len(struct)
    TOT = sum(sum(s) for s in struct)
    # flat column offsets per (slot, run)
    offs = []
    o = 0
    for s in range(S):
        ro = []
        for r in struct[s]:
            ro.append(o)
            o += r
        offs.append(ro)

    nc = bacc.Bacc()
    xg = nc.dram_tensor("xg", [P, N_KC, TOT], bf16, kind="ExternalInput")
    wv = nc.dram_tensor("wv", [1, TOT], f32, kind="ExternalInput")
    wgt = nc.dram_tensor("wgt", [S, P, N_KC, I], bf16, kind="ExternalInput")
    wut = nc.dram_tensor("wut", [S, P, N_KC, I], bf16, kind="ExternalInput")
    wdt = nc.dram_tensor("wdt", [S, P, N_IC, H], bf16, kind="ExternalInput")
    y = nc.dram_tensor("y", [P, N_KC, TOT], bf16, kind="ExternalOutput")

    # flatten runs; software-pipeline: gate/up of run j, then down of run j-1,
    # so the PE never idles (idle resets the p-state ramp to 2x cycle time).
    runs = []
    for s in range(S):
        for r, R in enumerate(struct[s]):
            runs.append((s, r, R, offs[s][r]))

    with tile.TileContext(nc) as tc:
        with tc.tile_pool(name="singles", bufs=1) as singles, \
             tc.tile_pool(name="wpool", bufs=2) as wpool, \
             tc.tile_pool(name="xpool", bufs=2) as xpool, \
             tc.tile_pool(name="hpool", bufs=2) as hpool, \
             tc.tile_pool(name="ypool", bufs=2) as ypool, \
             tc.tile_pool(name="pp", bufs=2, space="PSUM") as pp:
            w_tiles = {}
            state = {}  # pipeline state of the previous run
            queues = [nc.sync, nc.scalar, nc.gpsimd]
            KC3 = [(0, 6), (6, 11), (11, 16)]
            H3 = [(0, 640), (640, 1280), (1280, H)]

            # separate tile per DMA chunk: consumers depend on the chunk that
            # carries their slice, not on the whole-matrix load
            def emit_wg(s):
                ts = []
                for q, (a, b) in enumerate(KC3):
                    t = wpool.tile([P, b - a, I], bf16, name=f"wg_{s}_{q}",
                                   tag=f"wg{q}")
                    queues[(q + 1) % 3].dma_start(out=t, in_=wgt[s][:, a:b, :])
                    ts.append(((a, b), t))
                return ts

            def emit_wu(s):
                ts = []
                for q, (a, b) in enumerate(KC3):
                    t = wpool.tile([P, b - a, I], bf16, name=f"wu_{s}_{q}",
                                   tag=f"wu{q}")
                    queues[(q + 2) % 3].dma_start(out=t, in_=wut[s][:, a:b, :])
                    ts.append(((a, b), t))
                return ts

            def emit_wd(s):
                ts = []
                for q, (a, b) in enumerate(H3):
                    t = wpool.tile([P, N_IC, b - a], bf16, name=f"wd_{s}_{q}",
                                   tag=f"wd{q}")
                    queues[q].dma_start(out=t, in_=wdt[s][:, :, a:b])
                    ts.append(((a, b), t))
                return ts

            def emit_weights(s):
                w_tiles[s] = (emit_wg(s), emit_wu(s), emit_wd(s))

            def kc_slice(ts, kc):
                for (a, b), t in ts:
                    if a <= kc < b:
                        return t[:, kc - a, :]
                raise AssertionError

            def hc_slice(ts, ic, hc):
                for (a, b), t in ts:
                    if a <= hc * P < b:
                        return t[:, ic, hc * P - a:(hc + 1) * P - a]
                raise AssertionError

            def emit_down(prev, last=False):
                s, r, R, O = prev["run"]
                wd_t = w_tiles[s][2]
                h_t = prev["h"]
                YB = [(0, 6), (6, 11), (11, 16)]
                y_ts = [ypool.tile([P, b - a, RMAX], bf16, name=f"y_{s}_{r}_{q}",
                                   tag=f"y{q}")
                        for q, (a, b) in enumerate(YB)]
                for hc in range(N_HC):
                    pd = pp.tile([P, RMAX], f32, name=f"pd_{s}_{r}_{hc}",
                                 tag="pd", bufs=3)
                    for ic in range(N_IC):
                        nc.tensor.matmul(
                            pd[:, :R],
                            hc_slice(wd_t, ic, hc),
                            h_t[ic][:, :R],
                            start=(ic == 0), stop=(ic == N_IC - 1))
                    yq = 0 if hc < 6 else (1 if hc < 11 else 2)
                    ya = YB[yq][0]
                    if hc % 2 == 0:
                        nc.vector.tensor_copy(out=y_ts[yq][:, hc - ya, :R],
                                              in_=pd[:, :R])
                    else:
                        nc.scalar.activation(y_ts[yq][:, hc - ya, :R],
                                             pd[:, :R], AF.Copy)
                    if hc + 1 in (6, 11, 16):
                        a, b = YB[yq]
                        queues[yq].dma_start(out=y[:, a:b, O:O + R],
                                             in_=y_ts[yq][:, :, :R])

            def emit_xg(s, r, R, O):
                ts = []
                for q, (a, b) in enumerate(KC3):
                    t = xpool.tile([P, b - a, RMAX], bf16,
                                   name=f"xg_{s}_{r}_{q}", tag=f"xg{q}")
                    queues[q].dma_start(out=t[:, :, :R], in_=xg[:, a:b, O:O + R])
                    ts.append(((a, b), t))
                return ts

            # combine-weight vector, broadcast on device per run
            wv_t = singles.tile([1, TOT], f32, name="wv", tag="wv")
            nc.sync.dma_start(out=wv_t, in_=wv[:, :])

            # PE p-state warmup: a long dummy accumulation group keeps the PE
            # busy through the initial DMA wait so real matmuls start at full
            # clock (any idle gap resets the ramp)
            if WARM:
                wsb = singles.tile([P, P], bf16, name="warm_sb", tag="wsb")
                nc.vector.memset(wsb, 0.0)
                pdum = pp.tile([P, RMAX], f32, name="pdum", tag="pd", bufs=3)
                for i in range(WARM):
                    nc.tensor.matmul(pdum[:, :P], wsb, wsb,
                                     start=(i == 0), stop=(i == WARM - 1))

            # startup: stream gate operands in fine kc pieces, round-robin on
            # queues, xg piece adjacent to wg piece so the first matmuls can
            # begin as soon as the first pair lands
            s0, r0, R0, O0 = runs[0]
            x_tiles = {}
            xg0 = emit_xg(s0, r0, R0, O0)
            wg0 = emit_wg(0)
            w_tiles[0] = (wg0, emit_wu(0), emit_wd(0))
            x_tiles[0] = xg0

            # gate/up interleave tolerant of late wu arrival; first run is
            # all-gate kc-outer so matmuls start with the first piece pair
            GU = [(0, 'g'), (1, 'g'), (0, 'u'), (2, 'g'), (1, 'u'),
                  (3, 'g'), (2, 'u'), (3, 'u')]

            for j, (s, r, R, O) in enumerate(runs):
                if j not in x_tiles:
                    x_tiles[j] = emit_xg(s, r, R, O)
                xg_t = x_tiles.pop(j)
                # prefetch next slot's weights one full slot ahead
                if r == 0 and s + 1 < S:
                    emit_weights(s + 1)

                wb = hpool.tile([P, RMAX], f32, name=f"wb_{s}_{r}", tag="wb")
                nc.gpsimd.partition_broadcast(wb[:, :R], wv_t[:, O:O + R])

                wg_t, wu_t, _ = w_tiles[s]
                pg_t = [None] * N_IC
                h_t = [None] * N_IC
                for ic, which in GU:
                    if which == 'g':
                        pg = pp.tile([P, RMAX], f32, name=f"pg_{s}_{r}_{ic}",
                                     tag="pg", bufs=3)
                        for kc in range(N_KC):
                            nc.tensor.matmul(
                                pg[:, :R],
                                kc_slice(wg_t, kc)[:, ic * P:(ic + 1) * P],
                                kc_slice(xg_t, kc)[:, :R],
                                start=(kc == 0), stop=(kc == N_KC - 1))
                        sil = hpool.tile([P, RMAX], f32,
                                         name=f"sil_{s}_{r}_{ic}", tag="sil")
                        nc.scalar.activation(sil[:, :R], pg[:, :R], AF.Sigmoid)
                        silg = hpool.tile([P, RMAX], f32,
                                          name=f"silg_{s}_{r}_{ic}", tag="silg")
                        nc.vector.tensor_mul(silg[:, :R], sil[:, :R], pg[:, :R])
                        pg_t[ic] = silg
                    else:
                        pu = pp.tile([P, RMAX], f32, name=f"pu_{s}_{r}_{ic}",
                                     tag="pu")
                        for kc in range(N_KC):
                            nc.tensor.matmul(
                                pu[:, :R],
                                kc_slice(wu_t, kc)[:, ic * P:(ic + 1) * P],
                                kc_slice(xg_t, kc)[:, :R],
                                start=(kc == 0), stop=(kc == N_KC - 1))
                        silg = pg_t[ic]
                        uw = hpool.tile([P, RMAX], f32, name=f"uw_{s}_{r}_{ic}",
                                        tag="uw")
                        nc.vector.tensor_mul(uw[:, :R], silg[:, :R], pu[:, :R])
                        ht = hpool.tile([P, RMAX], bf16, name=f"h_{s}_{r}_{ic}",
                                        tag=f"h{ic}")
                        nc.vector.tensor_mul(ht[:, :R], uw[:, :R], wb[:, :R])
                        h_t[ic] = ht

                if state:
                    emit_down(state)
                state = {"run": (s, r, R, O), "h": h_t}
            emit_down(state, last=True)
    nc.finalize()
    return nc


def pack_inputs(struct, slots, x, W, weights):
    """Build per-core input maps (bf16)."""
    w_gate, w_up, w_down, ws_gate, ws_up, ws_down = weights
    S = len(struct)
    TOT = sum(sum(s) for s in struct)
    offs = []
    o = 0
    for s in range(S):
        ro = []
        for r in struct[s]:
            ro.append(o)
            o += r
        offs.append(ro)

    x = np.asarray(x, np.float32)
    # xTr[p, c, t] = x[t, c*128+p]
    xTr16 = np.ascontiguousarray(
        x.T.reshape(N_KC, P, T).transpose(1, 0, 2)).astype(BF16)

    wcache = {}

    def expert_w(e):
        if e not in wcache:
            if e == -1:
                g, u, d = ws_gate, ws_up, ws_down
            else:
                g, u, d = w_gate[e], w_up[e], w_down[e]
            wcache[e] = (
                np.ascontiguousarray(
                    g.reshape(N_KC, P, I).transpose(1, 0, 2)).astype(BF16),
                np.ascontiguousarray(
                    u.reshape(N_KC, P, I).transpose(1, 0, 2)).astype(BF16),
                np.ascontiguousarray(
                    d.reshape(N_IC, P, H).transpose(1, 0, 2)).astype(BF16),
            )
        return wcache[e]

    in_maps = []
    for c in range(N_CORES):
        xgv = np.zeros((P, N_KC, TOT), BF16)
        wvv = np.zeros((1, TOT), np.float32)
        wgv = np.zeros((S, P, N_KC, I), BF16)
        wuv = np.zeros((S, P, N_KC, I), BF16)
        wdv = np.zeros((S, P, N_IC, H), BF16)
        for s in range(S):
            ch = slots[c][s]
            if ch is None:
                continue
            e, idx = ch
            wgv[s], wuv[s], wdv[s] = expert_w(e)
            if e == -1:
                wts = np.ones(len(idx), np.float32)
            else:
                wts = W[idx, e] * np.float32(ROUTED_SCALING)
            pos = 0
            for r, R in enumerate(struct[s]):
                blk = idx[pos:pos + R]
                if len(blk) == 0:
                    break
                O = offs[s][r]
                xgv[:, :, O:O + len(blk)] = xTr16[:, :, blk]
                wvv[0, O:O + len(blk)] = wts[pos:pos + len(blk)]
                pos += len(blk)
        in_maps.append({"xg": xgv, "wv": wvv, "wgt": wgv, "wut": wuv,
                        "wdt": wdv})
    return in_maps


def combine(struct, slots, results):
    S = len(struct)
    offs = []
    o = 0
    for s in range(S):
        ro = []
        for r in struct[s]:
            ro.append(o)
            o += r
        offs.append(ro)
    out = np.zeros((T, H), np.float32)
    for c in range(N_CORES):
        yv = np.asarray(results[c]["y"], dtype=np.float32)  # [P, 16, TOT]
        yfull = yv.transpose(1, 0, 2).reshape(H, -1)        # [H, TOT]
        for s in range(S):
            ch = slots[c][s]
            if ch is None:
                continue
            _, idx = ch
            pos = 0
            for r, R in enumerate(struct[s]):
                blk = idx[pos:pos + R]
                if len(blk) == 0:
                    break
                O = offs[s][r]
                out[blk] += yfull[:, O:O + len(blk)].T
                pos += len(blk)
    return out


def prepare(**inputs):
    """Routing + planning + packing (everything except device execution)."""
    x = np.asarray(inputs["hidden_states"], np.float32)
    W = route_np(x, inputs["gate_w"], inputs["expert_bias"])
    struct, slots = make_plan(W)
    weights = tuple(
        np.asarray(inputs[k], np.float32)
        for k in ("w_gate", "w_up", "w_down", "ws_gate", "ws_up", "ws_down"))
    in_maps = pack_inputs(struct, slots, x, W, weights)
    return struct, slots, in_maps


def kernel(**inputs):
    from concourse.bass_utils import run_bass_kernel_spmd
    struct, slots, in_maps = prepare(**inputs)
    nc = build_program(struct)
    res = run_bass_kernel_spmd(nc, in_maps, core_ids=list(range(N_CORES)))
    return combine(struct, slots, res.results)
